# revision 1
# baseline (speedup 1.0000x reference)
import numpy as np
from contextlib import ExitStack

import concourse.bass as bass
import concourse.tile as tile
import concourse.mybir as mybir
from concourse import bacc
from concourse.bass_utils import run_bass_kernel_spmd

L = 2048
D = 1024
H = 16
HD = 64
N_CORES = 8
HPC = 2
SCALE = float(HD) ** -0.5

FP32 = mybir.dt.float32
FP16 = mybir.dt.float16
BF16 = mybir.dt.bfloat16
Alu = mybir.AluOpType
Act = mybir.ActivationFunctionType

N_BF16_ITERS = 9
N_F32_ITERS = 2
NEG_BIG = -1.0e30
MAX_INIT = -3.0e38

RB_PAIRS = [(0, 15), (4, 11), (1, 14), (5, 10), (2, 13), (6, 9), (3, 12), (7, 8)]
PAIR_W = 17 * 128


def _units_of_group(g):
    units = []
    p0, p1 = RB_PAIRS[2 * g], RB_PAIRS[2 * g + 1]
    for h in range(HPC):
        for pi_local, (ra, rb_) in enumerate((p0, p1)):
            slot = 2 * pi_local + h
            na = 128 * (ra + 1)
            units.append((ra, h, slot, 0))
            units.append((rb_, h, slot, na))
    return units


def build_program(n_groups=4, do_newton=True, do_avwo=True, debug_out=None):
    nc = bacc.Bacc("TRN2", target_bir_lowering=False, debug=False, num_devices=1)

    xT_d = nc.dram_tensor("xT", [D, L], FP32, kind="ExternalInput")
    wq_d = nc.dram_tensor("wqT", [D, 128], FP32, kind="ExternalInput")
    wk_d = nc.dram_tensor("wkT", [D, 128], FP32, kind="ExternalInput")
    wv_d = nc.dram_tensor("wvT", [D, 128], FP32, kind="ExternalInput")
    wo_d = nc.dram_tensor("woT", [128, D], FP32, kind="ExternalInput")
    mneg_d = nc.dram_tensor("mneg", [128, 128], FP32, kind="ExternalInput")
    m01_d = nc.dram_tensor("m01", [128, 128], FP32, kind="ExternalInput")
    ident_d = nc.dram_tensor("ident", [128, 128], FP32, kind="ExternalInput")
    out_d = nc.dram_tensor("out", [L, D], FP32, kind="ExternalOutput")

    with tile.TileContext(nc) as tc:
        with ExitStack() as ctx:
            persist = ctx.enter_context(tc.tile_pool(name="persist", bufs=1))
            qT = persist.tile([128, L], FP32, tag="qT")
            kT = persist.tile([128, L], FP32, tag="kT")
            vt = persist.tile([128, 16, 64 * HPC], FP16, tag="vt")
            woT = persist.tile([128, D], FP32, tag="woT")
            mneg = persist.tile([128, 128], FP32, tag="mneg")
            m01 = persist.tile([128, 128], FP32, tag="m01")
            ident = persist.tile([128, 128], FP32, tag="ident")
            ident_h = persist.tile([128, 128], FP16, tag="identh")
            zeros_bf = persist.tile([128, L], BF16, tag="zbf")
            trash_a = persist.tile([128, L], BF16, tag="tra")
            trash_d = persist.tile([128, L], BF16, tag="trd")
            trash_c = persist.tile([128, L], BF16, tag="trc")

            NST = 32

            def stat(tag):
                return persist.tile([128, NST], FP32, tag=tag, name=tag)

            maxF, maxD = stat("maxF"), stat("maxD")
            sumF, sumD = stat("sumF"), stat("sumD")
            mx, sm = stat("mx"), stat("sm")
            Tt, nT = stat("T"), stat("nT")
            Ft, Ct = stat("F"), stat("C")
            rec, Fm, dlt = stat("rec"), stat("Fm"), stat("dlt")
            tau, ntau = stat("tau"), stat("ntau")
            sump, rz = stat("sump"), stat("rz")
            nh = stat("nh")

            nc.sync.dma_start(mneg[:], mneg_d.ap())
            nc.sync.dma_start(m01[:], m01_d.ap())
            nc.sync.dma_start(ident[:], ident_d.ap())
            nc.scalar.copy(ident_h[:], ident[:])
            nc.sync.dma_start(woT[:], wo_d.ap())
            nc.vector.memset(zeros_bf[:], 0.0)
            nc.vector.memset(maxF[:], MAX_INIT)
            nc.vector.memset(sumF[:], 0.0)
            nc.vector.memset(maxD[:], MAX_INIT)
            nc.vector.memset(sumD[:], 0.0)
            for g in range(n_groups):
                for ui, (rb, h, slot, off) in enumerate(_units_of_group(g)):
                    col = 8 * g + ui
                    nc.vector.memset(nh[:, col:col + 1], 64.0 * (rb + 1))

            with ExitStack() as p1:
                ph1 = p1.enter_context(tc.tile_pool(name="ph1", bufs=1))
                ph1p = p1.enter_context(
                    tc.tile_pool(name="ph1p", bufs=2, space="PSUM"))
                xt = ph1.tile([128, 8, L], FP32, tag="xt")
                wqs = ph1.tile([128, 8, 128], FP32, tag="wqs")
                wks = ph1.tile([128, 8, 128], FP32, tag="wks")
                wvs = ph1.tile([128, 8, 128], FP32, tag="wvs")

                xview = xT_d.ap().rearrange("(c p) n -> p c n", p=128)
                for c in range(8):
                    nc.sync.dma_start(xt[:, c, :], xview[:, c, :])
                nc.sync.dma_start(wqs[:], wq_d.ap().rearrange("(c p) m -> p c m", p=128))
                nc.sync.dma_start(wks[:], wk_d.ap().rearrange("(c p) m -> p c m", p=128))
                nc.sync.dma_start(wvs[:], wv_d.ap().rearrange("(c p) m -> p c m", p=128))

                for dst, wsb in ((qT, wqs), (kT, wks)):
                    for ic in range(4):
                        ps = ph1p.tile([128, 512], FP32, tag="pp")
                        for e in range(8):
                            nc.tensor.matmul(
                                ps[:], wsb[:, e, :], xt[:, e, 512 * ic:512 * (ic + 1)],
                                start=(e == 0), stop=(e == 7))
                        if ic % 2 == 0:
                            nc.scalar.copy(dst[:, 512 * ic:512 * (ic + 1)], ps[:])
                        else:
                            nc.vector.tensor_copy(dst[:, 512 * ic:512 * (ic + 1)], ps[:])
                for jt in range(16):
                    ps = ph1p.tile([128, 512], FP32, tag="pp")
                    for e in range(8):
                        nc.tensor.matmul(
                            ps[:, :128], xt[:, e, 128 * jt:128 * (jt + 1)], wvs[:, e, :],
                            start=(e == 0), stop=(e == 7))
                    if jt % 2 == 0:
                        nc.scalar.copy(vt[:, jt, :], ps[:, :128])
                    else:
                        nc.vector.tensor_copy(vt[:, jt, :], ps[:, :128])

            if debug_out == "qkv":
                flat = out_d.ap().rearrange("a b -> (a b)")
                nc.sync.dma_start(flat[0:262144], qT[:])
                nc.sync.dma_start(flat[262144:524288], kT[:])

            s_pool = ctx.enter_context(tc.tile_pool(name="spair", bufs=2))
            sb_pool = ctx.enter_context(tc.tile_pool(name="sbpair", bufs=2))
            p_pool = ctx.enter_context(tc.tile_pool(name="ppair", bufs=1))
            ps_sc = ctx.enter_context(tc.tile_pool(name="ps_sc", bufs=1, space="PSUM"))
            ps_av = ctx.enter_context(tc.tile_pool(name="ps_av", bufs=2, space="PSUM"))
            ps_tr = ctx.enter_context(tc.tile_pool(name="ps_tr", bufs=2, space="PSUM"))
            ptb_pool = ctx.enter_context(tc.tile_pool(name="ptb", bufs=2))
            oc_pool = ctx.enter_context(tc.tile_pool(name="oc", bufs=2))
            wo_pool = ctx.enter_context(tc.tile_pool(name="woout", bufs=2))

            copy_flip = [0]

            def balanced_copy(dst, src):
                if copy_flip[0] % 2 == 0:
                    nc.scalar.copy(dst, src)
                else:
                    nc.vector.tensor_copy(dst, src)
                copy_flip[0] += 1

            for g in range(n_groups):
                units = _units_of_group(g)
                gsl = slice(8 * g, 8 * g + 8)
                hsl = slice(8 * g + 4, 8 * g + 8)
                Sg = [s_pool.tile([128, PAIR_W], FP32, tag=f"sp{s}", name=f"sp{s}_{g}")
                      for s in range(4)]
                Sbg = [sb_pool.tile([128, PAIR_W], BF16, tag=f"sb{s}", name=f"sb{s}_{g}")
                       for s in range(4)]

                for ui, (rb, h, slot, off) in enumerate(units):
                    col = 8 * g + ui
                    n = 128 * (rb + 1)
                    full = n - 128
                    S, Sb = Sg[slot], Sbg[slot]
                    ps = ps_sc.tile([128, 2048], FP32, tag="sc", name=f"sc{g}_{ui}")
                    for c0 in range(0, n, 512):
                        w = min(512, n - c0)
                        nc.tensor.matmul(
                            ps[:, c0:c0 + w],
                            qT[64 * h:64 * h + 64, 128 * rb:128 * rb + 128],
                            kT[64 * h:64 * h + 64, c0:c0 + w],
                            start=True, stop=True)
                    if full > 0:
                        nc.scalar.activation(
                            Sb[:, off:off + full], ps[:, :full], Act.Identity,
                            bias=0.0, accum_out=sumF[:, col:col + 1])
                        nc.vector.tensor_scalar(
                            out=S[:, off:off + full], in0=ps[:, :full],
                            scalar1=0.0, scalar2=MAX_INIT,
                            op0=Alu.add, op1=Alu.max,
                            accum_out=maxF[:, col:col + 1])
                    nc.vector.tensor_tensor(
                        S[:, off + full:off + n], ps[:, full:n], mneg[:], Alu.add)
                    nc.vector.tensor_scalar(
                        out=trash_c[:, :128], in0=S[:, off + full:off + n],
                        scalar1=0.0, scalar2=MAX_INIT,
                        op0=Alu.add, op1=Alu.max,
                        accum_out=maxD[:, col:col + 1])
                    nc.vector.scalar_tensor_tensor(
                        out=trash_d[:, :128],
                        in0=ps[:, full:n], scalar=1.0, in1=m01[:],
                        op0=Alu.mult, op1=Alu.mult,
                        accum_out=sumD[:, col:col + 1])
                    nc.vector.tensor_copy(Sb[:, off + full:off + n],
                                          S[:, off + full:off + n])

                nc.vector.tensor_tensor(mx[:, gsl], maxF[:, gsl], maxD[:, gsl], Alu.max)
                nc.vector.tensor_tensor(sm[:, gsl], sumF[:, gsl], sumD[:, gsl], Alu.add)
                nc.vector.tensor_scalar_add(Tt[:, gsl], mx[:, gsl], -1.0)
                nc.vector.tensor_scalar(
                    out=nT[:, gsl], in0=mx[:, gsl], scalar1=-1.0, scalar2=1.0,
                    op0=Alu.mult, op1=Alu.add)

                if debug_out == "scores":
                    flat2 = out_d.ap().rearrange("a b -> (a b)")
                    for slot in range(4):
                        nc.sync.dma_start(
                            flat2[278528 * slot:278528 * (slot + 1)], Sg[slot][:])
                    continue
                if not do_newton:
                    continue

                def emit_passes(use_bf16, skip_F=False):
                    for ui, (rb, h, slot, off) in enumerate(units):
                        col = 8 * g + ui
                        n = 128 * (rb + 1)
                        Ssrc = Sbg[slot] if use_bf16 else Sg[slot]
                        if ui < 4:
                            if not skip_F:
                                nc.scalar.activation(
                                    trash_a[:, :n], Ssrc[:, off:off + n], Act.Relu,
                                    bias=nT[:, col:col + 1],
                                    accum_out=Ft[:, col:col + 1])
                            nc.vector.tensor_scalar(
                                out=trash_c[:, :n], in0=Ssrc[:, off:off + n],
                                scalar1=Tt[:, col:col + 1], scalar2=0.0,
                                op0=Alu.is_gt, op1=Alu.add,
                                accum_out=Ct[:, col:col + 1])
                        else:
                            if not skip_F:
                                nc.vector.scalar_tensor_tensor(
                                    out=trash_d[:, :n], in0=Ssrc[:, off:off + n],
                                    scalar=nT[:, col:col + 1], in1=zeros_bf[:, :n],
                                    op0=Alu.add, op1=Alu.max,
                                    accum_out=Ft[:, col:col + 1])
                            nc.scalar.activation(
                                trash_a[:, :n], Ssrc[:, off:off + n], Act.Sign,
                                bias=nT[:, col:col + 1],
                                accum_out=Ct[:, col:col + 1])
                    nc.vector.scalar_tensor_tensor(
                        out=Ct[:, hsl], in0=Ct[:, hsl], scalar=0.5, in1=nh[:, hsl],
                        op0=Alu.mult, op1=Alu.add)

                def newton_update():
                    nc.vector.tensor_scalar_max(Ct[:, gsl], Ct[:, gsl], 1.0)
                    nc.vector.reciprocal(rec[:, gsl], Ct[:, gsl])
                    nc.vector.tensor_scalar_add(Fm[:, gsl], Ft[:, gsl], -1.0)
                    nc.vector.tensor_tensor(dlt[:, gsl], Fm[:, gsl], rec[:, gsl], Alu.mult)
                    nc.vector.tensor_tensor(Tt[:, gsl], Tt[:, gsl], dlt[:, gsl], Alu.add)
                    nc.vector.tensor_tensor(nT[:, gsl], nT[:, gsl], dlt[:, gsl], Alu.subtract)

                for _ in range(N_BF16_ITERS):
                    emit_passes(True)
                    newton_update()
                for _ in range(N_F32_ITERS):
                    emit_passes(False)
                    newton_update()

                emit_passes(False, skip_F=True)
                nc.vector.tensor_scalar_max(Ct[:, gsl], Ct[:, gsl], 1.0)
                nc.vector.reciprocal(rec[:, gsl], Ct[:, gsl])
                nc.vector.tensor_scalar_add(Fm[:, gsl], sm[:, gsl], -1.0)
                nc.vector.tensor_tensor(tau[:, gsl], Fm[:, gsl], rec[:, gsl], Alu.mult)
                nc.vector.tensor_scalar_mul(ntau[:, gsl], tau[:, gsl], -1.0)

                if not do_avwo:
                    continue

                Pg = [p_pool.tile([128, PAIR_W], FP16, tag=f"pp{s}", name=f"pp{s}_{g}")
                      for s in range(4)]
                outc_of_rb = {}
                for ui, (rb, h, slot, off) in enumerate(units):
                    col = 8 * g + ui
                    n = 128 * (rb + 1)
                    S, P = Sg[slot], Pg[slot]
                    nc.scalar.activation(
                        P[:, off:off + n], S[:, off:off + n], Act.Relu,
                        bias=ntau[:, col:col + 1],
                        accum_out=sump[:, col:col + 1])
                    nc.vector.tensor_scalar_add(
                        Fm[:, col:col + 1], sump[:, col:col + 1], 1.0e-10)
                    nc.vector.reciprocal(rz[:, col:col + 1], Fm[:, col:col + 1])

                    if rb not in outc_of_rb:
                        outc_of_rb[rb] = oc_pool.tile(
                            [128, 128], FP32, tag=f"oc{ui % 2}", name=f"oc{g}_{rb}")
                    outc = outc_of_rb[rb]

                    av = ps_av.tile([128, 512], FP32, tag="av", name=f"av{g}_{ui}")
                    nt = n // 128
                    for c0 in range(0, nt, 4):
                        cw = min(4, nt - c0)
                        pt_ps = ps_tr.tile([128, 512], FP16, tag="tr",
                                           name=f"ptp{g}_{ui}_{c0}")
                        for c in range(cw):
                            jt = c0 + c
                            nc.tensor.transpose(
                                pt_ps[:, 128 * c:128 * (c + 1)],
                                P[:, off + 128 * jt:off + 128 * (jt + 1)],
                                ident_h[:])
                        pt_sb = ptb_pool.tile([128, 512], FP16, tag="ptb",
                                              name=f"ptb{g}_{ui}_{c0}")
                        balanced_copy(pt_sb[:, :128 * cw], pt_ps[:, :128 * cw])
                        for c in range(cw):
                            jt = c0 + c
                            nc.tensor.matmul(
                                av[:, :64], pt_sb[:, 128 * c:128 * (c + 1)],
                                vt[:, jt, 64 * h:64 * h + 64],
                                start=(jt == 0), stop=(jt == nt - 1))
                    nc.scalar.activation(
                        outc[:, 64 * h:64 * h + 64], av[:, :64], Act.Copy,
                        bias=0.0, scale=rz[:, col:col + 1])

                    if h == 1:
                        wo_out = wo_pool.tile([128, D], FP32, tag="wod",
                                              name=f"wod{g}_{rb}")
                        otb = ptb_pool.tile([128, 512], FP32, tag="otb",
                                            name=f"otb{g}_{rb}")
                        wps_t = ps_av.tile([128, 512], FP32, tag="av",
                                           name=f"ot{g}_{rb}")
                        nc.tensor.transpose(wps_t[:, :128], outc[:], ident[:])
                        balanced_copy(otb[:, :128], wps_t[:, :128])
                        for oc2 in range(2):
                            wps = ps_av.tile([128, 512], FP32, tag="av",
                                             name=f"wo{g}_{rb}_{oc2}")
                            nc.tensor.matmul(
                                wps[:], otb[:, :128], woT[:, 512 * oc2:512 * (oc2 + 1)],
                                start=True, stop=True)
                            balanced_copy(wo_out[:, 512 * oc2:512 * (oc2 + 1)], wps[:])
                        nc.sync.dma_start(
                            out_d.ap()[128 * rb:128 * (rb + 1), :], wo_out[:])

    nc.compile()
    return nc


_CACHE = {}


def _get_nc():
    if "nc" not in _CACHE:
        _CACHE["nc"] = build_program()
    return _CACHE["nc"]


def _host_inputs(x, Wq, Wk, Wv, Wo):
    xT = np.ascontiguousarray(x[0].T).astype(np.float32)
    ii = np.arange(128)
    mneg = np.where(ii[None, :] > ii[:, None], np.float32(NEG_BIG),
                    np.float32(0.0)).astype(np.float32)
    m01 = (ii[None, :] <= ii[:, None]).astype(np.float32)
    in_maps = []
    for c in range(N_CORES):
        hsl = slice(128 * c, 128 * (c + 1))
        in_maps.append({
            "xT": xT,
            "wqT": np.ascontiguousarray((Wq[hsl] * np.float32(SCALE)).T).astype(np.float32),
            "wkT": np.ascontiguousarray(Wk[hsl].T).astype(np.float32),
            "wvT": np.ascontiguousarray(Wv[hsl].T).astype(np.float32),
            "woT": np.ascontiguousarray(Wo[:, hsl].T).astype(np.float32),
            "mneg": mneg,
            "m01": m01,
            "ident": np.eye(128, dtype=np.float32),
        })
    return in_maps


def kernel(x, Wq, Wk, Wv, Wo, _trace=False):
    nc = _get_nc()
    in_maps = _host_inputs(np.asarray(x), np.asarray(Wq), np.asarray(Wk),
                           np.asarray(Wv), np.asarray(Wo))
    res = run_bass_kernel_spmd(nc, in_maps, core_ids=list(range(N_CORES)),
                               trace=_trace)
    out = np.zeros((L, D), np.float32)
    for c in range(N_CORES):
        out += res.results[c]["out"]
    if _trace:
        _CACHE["last_results"] = res
    return out.reshape(1, L, D)

